# revision 1
# baseline (speedup 1.0000x reference)
"""BlockGRU Trainium2 kernel.

Block-diagonal GRU cell: 8 independent blocks (block_size 256), batch 2048,
input_dim 1024. Sharded one block per NeuronCore (8 cores).

Per-core layout: gates on partitions, batch on the free dimension
(everything transposed on the host, which is free). Matmul operands are
fp16 (measured end-to-end rel-L2 error vs the fp32 reference: 2.6e-4;
fp16 halves the DMA streams and runs the PE at full rate with fast
weight load); accumulation and all elementwise math stay fp32. r/z gate
pre-activations accumulate input-projection + hidden-projection directly
in PSUM; i_n and h_n are kept separate for the r-gating. Per-partition
biases fuse into ScalarE activation ops (sigmoid/tanh) and a
scalar_tensor_tensor on VectorE; 1-z runs on the idle GPSIMD engine.
"""

import sys

if "/opt/trn_rl_repo" not in sys.path:
    sys.path.insert(0, "/opt/trn_rl_repo")

import numpy as np

INPUT_DIM = 1024
HIDDEN_DIM = 2048
NUM_BLOCKS = 8
BS = HIDDEN_DIM // NUM_BLOCKS  # 256
G3 = 3 * BS                    # 768
BATCH = 2048
CHUNKS = [512, 512, 512, 256, 256]   # batch chunks (PSUM bank = 512 fp32;
                                     # small tail chunks shorten the post-PE tail)
KX = INPUT_DIM // 128          # 8 contraction tiles on the input side
KH = BS // 128                 # 2 contraction tiles on the hidden side
ST = BS // 128                 # 2 state partition-tiles per block

_cached = None


def _build():
    import concourse.tile as tile
    import concourse.mybir as mybir
    from concourse import bacc

    f32 = mybir.dt.float32
    f16 = mybir.dt.float16
    ALU = mybir.AluOpType
    ACT = mybir.ActivationFunctionType

    nc = bacc.Bacc("TRN2", target_bir_lowering=False, debug=False, num_devices=8)

    xT = nc.dram_tensor("xT", [INPUT_DIM, BATCH], f16, kind="ExternalInput")
    wih = nc.dram_tensor("wih", [INPUT_DIM, G3], f16, kind="ExternalInput")
    whh = nc.dram_tensor("whh", [BS, G3], f16, kind="ExternalInput")
    hT = nc.dram_tensor("hT", [BS, BATCH], f16, kind="ExternalInput")
    bias = nc.dram_tensor("bias", [128, 5 * ST], f32, kind="ExternalInput")
    oT = nc.dram_tensor("oT", [BS, BATCH], f32, kind="ExternalOutput")

    with tile.TileContext(nc) as tc:
        with (
            tc.tile_pool(name="const", bufs=1) as cp,
            tc.tile_pool(name="xin", bufs=3) as xp,
            tc.tile_pool(name="hin", bufs=3) as hp,
            tc.tile_pool(name="gates", bufs=4) as gp,
            tc.tile_pool(name="outs", bufs=3) as op,
            tc.tile_pool(name="psum", bufs=1, space="PSUM") as pp,
        ):
            # PE warm-up: harmless matmuls on a zeroed tile while the prefill
            # DMA runs, so the clock ramp (cold -> full rate) completes before
            # real work arrives. Uses the p0 PSUM slot ahead of chunk 0.
            wu = cp.tile([128, 32], f16, tag="wu")
            nc.vector.memset(wu[:], 0.0)
            pdummy = pp.tile([128, 32], f32, tag="p0", name="pdummy")
            for _ in range(48):
                nc.tensor.matmul(pdummy[0:32, :], wu[:, 0:32], wu[:],
                                 start=True, stop=True)

            # --- DMA prologue. The DMA queue is serial at HBM bandwidth, so
            # emission order == arrival order == PE consumption order: the
            # x-side weights and chunk-0 columns first (bulk of PE work),
            # hidden-side weights/state + biases after (consumed at the end
            # of chunk 0's accumulation). ---
            c0w = CHUNKS[0]
            cs0 = slice(0, c0w)
            wih_sb = []
            x0_t = []
            # k-tiles load pairwise-merged via 3D access patterns: fewer
            # DMA descriptors means the serial DMA stream outpaces PE's
            # k-major consumption, so chunk 0 runs stall-free.
            for kp in range(0, KX, 2):
                wm = cp.tile([128, 2 * G3], f16, tag=f"wih{kp}", name=f"wihm{kp}")
                nc.sync.dma_start(
                    wm[:].rearrange("p (k g) -> p k g", k=2),
                    wih.ap()[kp * 128:(kp + 2) * 128, :]
                        .rearrange("(k p) g -> p k g", p=128))
                wih_sb.append(wm[:, 0:G3])
                wih_sb.append(wm[:, G3:2 * G3])
                xm = xp.tile([128, 2 * c0w], f16, tag=f"x{kp}", name=f"xm{kp}")
                nc.sync.dma_start(
                    xm[:].rearrange("p (k c) -> p k c", k=2),
                    xT.ap()[kp * 128:(kp + 2) * 128, cs0]
                        .rearrange("(k p) b -> p k b", p=128))
                x0_t.append(xm[:, 0:c0w])
                x0_t.append(xm[:, c0w:2 * c0w])
                if kp == 4:
                    bias_sb = cp.tile([128, 5 * ST], f32, tag="bias")
                    nc.sync.dma_start(bias_sb[:], bias.ap())
            brz_sb = bias_sb[:, 0:2 * ST]
            bzn_sb = bias_sb[:, 2 * ST:3 * ST]
            bin_sb = bias_sb[:, 3 * ST:4 * ST]
            bhn_sb = bias_sb[:, 4 * ST:5 * ST]
            whm = cp.tile([128, 2 * G3], f16, tag="whm")
            nc.sync.dma_start(
                whm[:].rearrange("p (k g) -> p k g", k=2),
                whh.ap().rearrange("(k p) g -> p k g", p=128))
            whh_sb = [whm[:, 0:G3], whm[:, G3:2 * G3]]
            h0m = hp.tile([128, 2 * c0w], f16, tag="h0m")
            nc.sync.dma_start(
                h0m[:].rearrange("p (k c) -> p k c", k=2),
                hT.ap()[:, cs0].rearrange("(k p) b -> p k b", p=128))
            h0_t = [h0m[:, 0:c0w], h0m[:, c0w:2 * c0w]]

            cstart = 0
            for c, cw in enumerate(CHUNKS):
                cs = slice(cstart, cstart + cw)
                cstart += cw
                if c == 0:
                    x_t, h_t = x0_t, h0_t
                else:
                    x_t = []
                    for kp in range(0, KX, 2):
                        xm2 = xp.tile([128, 2 * cw], f16, tag=f"x{kp}",
                                      name=f"xc{kp}")
                        nc.sync.dma_start(
                            xm2[:].rearrange("p (k c) -> p k c", k=2),
                            xT.ap()[kp * 128:(kp + 2) * 128, cs]
                                .rearrange("(k p) b -> p k b", p=128))
                        x_t.append(xm2[:, 0:cw])
                        x_t.append(xm2[:, cw:2 * cw])
                    hm2 = hp.tile([128, 2 * cw], f16, tag="h0m", name="hc")
                    nc.sync.dma_start(
                        hm2[:].rearrange("p (k c) -> p k c", k=2),
                        hT.ap()[:, cs].rearrange("(k p) b -> p k b", p=128))
                    h_t = [hm2[:, 0:cw], hm2[:, cw:2 * cw]]

                # PSUM accumulators. r/z gates take input-proj + hidden-proj
                # into the same bank (only their sum is needed downstream).
                p_rz = [pp.tile([128, cw], f32, tag=f"p{gt}", name=f"prz{gt}")
                        for gt in range(2 * ST)]
                p_in = [pp.tile([128, cw], f32, tag=f"p{2 * ST + t_}", name=f"pin{t_}")
                        for t_ in range(ST)]
                p_hn = [pp.tile([128, cw], f32, tag=f"p{3 * ST + t_}", name=f"phn{t_}")
                        for t_ in range(ST)]

                # Input-side first, k-major, so PE consumption tracks the DMA
                # arrival order (wih[k]/x[k] pairs).  The last x k-tile plus
                # all hidden-side matmuls form per-psum "tail groups" ordered
                # so psums complete staggered: r-gates first (sigmoids start
                # draining banks early), i_n last (shortest post-PE chain).
                def gsl(gt):
                    return slice(gt * 128, (gt + 1) * 128)

                for k in range(KX - 1):
                    for gt in range(2 * ST):
                        nc.tensor.matmul(p_rz[gt][:], wih_sb[k][:, gsl(gt)],
                                         x_t[k][:], start=(k == 0), stop=False)
                    for t_ in range(ST):
                        nc.tensor.matmul(p_in[t_][:], wih_sb[k][:, gsl(4 + t_)],
                                         x_t[k][:], start=(k == 0), stop=False)
                kl = KX - 1
                last = (c == len(CHUNKS) - 1)
                o = op.tile([128, ST * cw], f32, tag="o")

                def r_tail(t_):
                    nc.tensor.matmul(p_rz[t_][:], wih_sb[kl][:, gsl(t_)],
                                     x_t[kl][:], start=False, stop=False)
                    for k in range(KH):
                        nc.tensor.matmul(p_rz[t_][:], whh_sb[k][:, gsl(t_)],
                                         h_t[k][:], start=False, stop=(k == KH - 1))

                def hn_tail(t_):
                    for k in range(KH):
                        nc.tensor.matmul(p_hn[t_][:], whh_sb[k][:, gsl(4 + t_)],
                                         h_t[k][:], start=(k == 0), stop=(k == KH - 1))

                def in_tail(t_):
                    nc.tensor.matmul(p_in[t_][:], wih_sb[kl][:, gsl(4 + t_)],
                                     x_t[kl][:], start=False, stop=True)

                def z_tail(t_):
                    gt = ST + t_
                    nc.tensor.matmul(p_rz[gt][:], wih_sb[kl][:, gsl(gt)],
                                     x_t[kl][:], start=False, stop=False)
                    for k in range(KH):
                        nc.tensor.matmul(p_rz[gt][:], whh_sb[k][:, gsl(gt)],
                                         h_t[k][:], start=False, stop=(k == KH - 1))

                def ew_r(t_):
                    r = gp.tile([128, cw], f32, tag=f"r{t_}", name=f"r{t_}")
                    nc.scalar.activation(r[:], p_rz[t_][:], ACT.Sigmoid,
                                         bias=brz_sb[:, t_:t_ + 1])
                    a = gp.tile([128, cw], f32, tag=f"a{t_}", name=f"a{t_}")
                    nc.vector.scalar_tensor_tensor(
                        a[:], p_hn[t_][:], bhn_sb[:, t_:t_ + 1], r[:],
                        ALU.add, ALU.mult)
                    return a

                def ew_z(t_):
                    z = gp.tile([128, cw], f32, tag=f"z{t_}", name=f"z{t_}")
                    nc.scalar.activation(z[:], p_rz[ST + t_][:], ACT.Sigmoid,
                                         bias=brz_sb[:, ST + t_:ST + t_ + 1])
                    zc = gp.tile([128, cw], f32, tag=f"zc{t_}", name=f"zc{t_}")
                    nc.gpsimd.tensor_scalar(zc[:], z[:], -1.0, 1.0,
                                            ALU.mult, ALU.add)
                    return z, zc

                def ew_zh(t_, z):
                    zh = gp.tile([128, cw], f32, tag=f"zh{t_}", name=f"zh{t_}")
                    nc.vector.tensor_mul(zh[:], z[:], h_t[t_][:])
                    return zh

                def ew_tanh(t_, a):
                    b2 = gp.tile([128, cw], f32, tag=f"b{t_}", name=f"b{t_}")
                    nc.vector.tensor_add(b2[:], a[:], p_in[t_][:])
                    n_ = gp.tile([128, cw], f32, tag=f"n{t_}", name=f"n{t_}")
                    nc.scalar.activation(n_[:], b2[:], ACT.Tanh,
                                         bias=bin_sb[:, t_:t_ + 1])
                    return n_

                def ew_out(t_, n_, zc, zh):
                    e = gp.tile([128, cw], f32, tag=f"e{t_}", name=f"e{t_}")
                    nc.vector.tensor_mul(e[:], n_[:], zc[:])
                    nc.vector.tensor_add(o[:, t_ * cw:(t_ + 1) * cw], e[:],
                                         zh[:])

                if not last:
                    # staggered psum completion: r-gates first (sigmoids free
                    # banks for the next chunk), i_n last (short post chain)
                    for t_ in range(ST):
                        r_tail(t_)
                    for t_ in range(ST):
                        hn_tail(t_)
                    for t_ in range(ST):
                        z_tail(t_)
                    for t_ in range(ST):
                        in_tail(t_)
                    as_ = [ew_r(t_) for t_ in range(ST)]
                    zzc = [ew_z(t_) for t_ in range(ST)]
                    zhs = [ew_zh(t_, zzc[t_][0]) for t_ in range(ST)]
                    ns_ = [ew_tanh(t_, as_[t_]) for t_ in range(ST)]
                    for t_ in range(ST):
                        ew_out(t_, ns_[t_], zzc[t_][1], zhs[t_])
                    nc.scalar.dma_start(
                        oT.ap().rearrange("(t p) b -> p t b", p=128)[:, :, cs],
                        o[:].rearrange("p (t c) -> p t c", t=ST))
                else:
                    # final chunk: i_n psums complete before the z-gates so
                    # the b2/tanh chain runs under the last matmuls; b2 goes
                    # ahead of zh on the VectorE queue; per-tile output DMAs
                    # on the scalar and sync DGE queues.
                    for t_ in range(ST):
                        r_tail(t_)
                    for t_ in range(ST):
                        hn_tail(t_)
                    for t_ in range(ST):
                        in_tail(t_)
                    for t_ in range(ST):
                        z_tail(t_)
                    as_ = [ew_r(t_) for t_ in range(ST)]
                    zzc = [ew_z(t_) for t_ in range(ST)]
                    ns_ = [ew_tanh(t_, as_[t_]) for t_ in range(ST)]
                    zhs = [ew_zh(t_, zzc[t_][0]) for t_ in range(ST)]
                    for t_ in range(ST):
                        ew_out(t_, ns_[t_], zzc[t_][1], zhs[t_])
                        eng = nc.scalar if t_ == 0 else nc.sync
                        eng.dma_start(
                            oT.ap()[t_ * 128:(t_ + 1) * 128, cs],
                            o[:, t_ * cw:(t_ + 1) * cw])

    nc.compile()
    return nc


def _get_nc():
    global _cached
    if _cached is None:
        _cached = _build()
    return _cached


def kernel(input, hidden, W_ih, W_hh, b_ih, b_hh):
    input = np.asarray(input, dtype=np.float32)
    hidden = np.asarray(hidden, dtype=np.float32)
    W_ih = np.asarray(W_ih, dtype=np.float32)
    W_hh = np.asarray(W_hh, dtype=np.float32)
    b_ih = np.asarray(b_ih, dtype=np.float32)
    b_hh = np.asarray(b_hh, dtype=np.float32)

    nc = _get_nc()
    from concourse.bass_utils import run_bass_kernel_spmd

    xT = np.ascontiguousarray(input.T.astype(np.float16))
    in_maps = []
    for n in range(NUM_BLOCKS):
        brz_n = (b_ih[n, :2 * BS] + b_hh[n, :2 * BS]).reshape(2 * ST, 128).T
        bzn_n = -brz_n[:, ST:]
        bin_n = b_ih[n, 2 * BS:].reshape(ST, 128).T
        bhn_n = b_hh[n, 2 * BS:].reshape(ST, 128).T
        bias_n = np.concatenate([brz_n, bzn_n, bin_n, bhn_n], axis=1)
        in_maps.append({
            "xT": xT,
            "wih": np.ascontiguousarray(W_ih[n].T.astype(np.float16)),
            "whh": np.ascontiguousarray(W_hh[n].T.astype(np.float16)),
            "hT": np.ascontiguousarray(hidden[:, n * BS:(n + 1) * BS].T.astype(np.float16)),
            "bias": np.ascontiguousarray(bias_n),
        })

    res = run_bass_kernel_spmd(nc, in_maps, core_ids=list(range(NUM_BLOCKS)))
    out = np.empty((BATCH, HIDDEN_DIM), dtype=np.float32)
    for n in range(NUM_BLOCKS):
        out[:, n * BS:(n + 1) * BS] = res.results[n]["oT"].T
    return out



# revision 2
# speedup vs baseline: 1.0750x; 1.0750x over previous
"""BlockGRU Trainium2 kernel.

Block-diagonal GRU cell: 8 independent blocks (block_size 256), batch 2048,
input_dim 1024. Sharded one block per NeuronCore (8 cores).

Per-core layout: gates on partitions, batch on the free dimension
(everything transposed on the host, which is free). Matmul datapath runs
fp8e4m3 with DoubleRow perf mode (two 128-deep k-slices contracted per
instruction at 0.5 cycles/row — 4x the fp16 rate). Accuracy is recovered
with error-compensation passes: activations and weights are split into
fp8 hi + fp8 residual streams (host-side quantization), and the
gate-sensitive z/n input projections accumulate the w_hi@x_lo and
w_lo@x_hi cross terms (the r gate and the short hidden-side projections
tolerate plain fp8; measured end-to-end rel-L2 vs the fp32 reference:
9.2e-3). Weights are pre-scaled by 64 into fp8's normal range; the
descale folds into the ScalarE activation `scale` and a host-prescaled
b_hn. The hidden state is shipped twice: fp8 for the matmul, fp16 for
the elementwise z*h path. PSUM accumulates everything in fp32;
per-partition biases fuse into ScalarE activation ops (sigmoid/tanh)
and a scalar_tensor_tensor on VectorE; 1-z runs on the Pool engine.
"""

import sys

if "/opt/trn_rl_repo" not in sys.path:
    sys.path.insert(0, "/opt/trn_rl_repo")

import numpy as np
import ml_dtypes

INPUT_DIM = 1024
HIDDEN_DIM = 2048
NUM_BLOCKS = 8
BS = HIDDEN_DIM // NUM_BLOCKS  # 256
G3 = 3 * BS                    # 768
GZN = 2 * BS                   # 512 z+n gate columns carrying residual passes
BATCH = 2048
CHUNKS = [512, 512, 512, 256, 256]   # batch chunks (PSUM bank = 512 fp32;
                                     # small tail chunks shorten the post-PE tail)
KP = INPUT_DIM // 256          # 4 DoubleRow k-pairs on the input side
ST = BS // 128                 # 2 state partition-tiles per block
WS = 64.0                      # weight pre-scale into fp8 normal range

_cached = None


def _build():
    import concourse.tile as tile
    import concourse.mybir as mybir
    from concourse import bacc

    f32 = mybir.dt.float32
    f16 = mybir.dt.float16
    f8 = mybir.dt.float8e4
    ALU = mybir.AluOpType
    ACT = mybir.ActivationFunctionType
    DR = mybir.MatmulPerfMode.DoubleRow

    nc = bacc.Bacc("TRN2", target_bir_lowering=False, debug=False, num_devices=8)

    xhiT = nc.dram_tensor("xhiT", [INPUT_DIM, BATCH], f8, kind="ExternalInput")
    xloT = nc.dram_tensor("xloT", [INPUT_DIM, BATCH], f8, kind="ExternalInput")
    wihHi = nc.dram_tensor("wihHi", [INPUT_DIM, G3], f8, kind="ExternalInput")
    wihLo = nc.dram_tensor("wihLo", [INPUT_DIM, GZN], f8, kind="ExternalInput")
    whh = nc.dram_tensor("whh", [BS, G3], f8, kind="ExternalInput")
    h8T = nc.dram_tensor("h8T", [BS, BATCH], f8, kind="ExternalInput")
    h16T = nc.dram_tensor("h16T", [BS, BATCH], f16, kind="ExternalInput")
    bias = nc.dram_tensor("bias", [128, 5 * ST], f32, kind="ExternalInput")
    oT = nc.dram_tensor("oT", [BS, BATCH], f32, kind="ExternalOutput")

    def pair3(t, inner):
        # [128, 2*inner] tile -> [128, 2, inner] DoubleRow AP (k-pair on dim1)
        return t[:].rearrange("p (k g) -> p k g", k=2, g=inner)

    with tile.TileContext(nc) as tc:
        with (
            tc.tile_pool(name="const", bufs=1) as cp,
            tc.tile_pool(name="xin", bufs=3) as xp,
            tc.tile_pool(name="hin", bufs=3) as hp,
            tc.tile_pool(name="gates", bufs=4) as gp,
            tc.tile_pool(name="outs", bufs=3) as op,
            tc.tile_pool(name="psum", bufs=1, space="PSUM") as pp,
        ):
            # PE warm-up: harmless matmuls on a zeroed tile while the prefill
            # DMA runs, so the clock ramp (cold -> full rate) completes before
            # real work arrives. Uses the p0 PSUM slot ahead of chunk 0.
            wu = cp.tile([128, 32], f16, tag="wu")
            nc.vector.memset(wu[:], 0.0)
            pdummy = pp.tile([128, 32], f32, tag="p0", name="pdummy")
            for _ in range(48):
                nc.tensor.matmul(pdummy[0:32, :], wu[:, 0:32], wu[:],
                                 start=True, stop=True)

            # --- DMA prologue on the sync (SP) queue. The queue is serial at
            # HBM bandwidth, so emission order == arrival order == PE
            # consumption order: whi/xhi k-pairs first (phase-1 matmuls),
            # then wlo (phase 2), xlo (phase 3), hidden-side + biases last
            # (consumed at the chunk-0 tail). ---
            c0w = CHUNKS[0]
            cs0 = slice(0, c0w)
            whi_sb = []
            xhi0 = []
            for kp in range(KP):
                wm = cp.tile([128, 2 * G3], f8, tag=f"whi{kp}", name=f"whi{kp}")
                nc.sync.dma_start(
                    pair3(wm, G3),
                    wihHi.ap()[kp * 256:(kp + 1) * 256, :]
                        .rearrange("(k p) g -> p k g", p=128))
                whi_sb.append(wm)
                xm = xp.tile([128, 2 * c0w], f8, tag=f"x{kp}", name=f"xhi0_{kp}")
                nc.sync.dma_start(
                    pair3(xm, c0w),
                    xhiT.ap()[kp * 256:(kp + 1) * 256, cs0]
                        .rearrange("(k p) b -> p k b", p=128))
                xhi0.append(xm)
            wlo_sb = []
            for kp in range(KP):
                wm = cp.tile([128, 2 * GZN], f8, tag=f"wlo{kp}", name=f"wlo{kp}")
                nc.sync.dma_start(
                    pair3(wm, GZN),
                    wihLo.ap()[kp * 256:(kp + 1) * 256, :]
                        .rearrange("(k p) g -> p k g", p=128))
                wlo_sb.append(wm)
            xlo0 = []
            for kp in range(KP):
                xm = xp.tile([128, 2 * c0w], f8, tag=f"xl{kp}", name=f"xlo0_{kp}")
                nc.sync.dma_start(
                    pair3(xm, c0w),
                    xloT.ap()[kp * 256:(kp + 1) * 256, cs0]
                        .rearrange("(k p) b -> p k b", p=128))
                xlo0.append(xm)
            whm = cp.tile([128, 2 * G3], f8, tag="whm")
            nc.sync.dma_start(
                pair3(whm, G3),
                whh.ap().rearrange("(k p) g -> p k g", p=128))
            h80 = hp.tile([128, 2 * c0w], f8, tag="h8", name="h80")
            nc.sync.dma_start(
                pair3(h80, c0w),
                h8T.ap()[:, cs0].rearrange("(k p) b -> p k b", p=128))
            bias_sb = cp.tile([128, 5 * ST], f32, tag="bias")
            nc.sync.dma_start(bias_sb[:], bias.ap())
            h160 = hp.tile([128, 2 * c0w], f16, tag="h16", name="h160")
            nc.sync.dma_start(
                pair3(h160, c0w),
                h16T.ap()[:, cs0].rearrange("(k p) b -> p k b", p=128))

            brz_sb = bias_sb[:, 0:2 * ST]
            bin_sb = bias_sb[:, 3 * ST:4 * ST]
            bhn_sb = bias_sb[:, 4 * ST:5 * ST]   # pre-scaled by WS on host

            def gsl(gt):
                return slice(gt * 128, (gt + 1) * 128)

            cstart = 0
            for c, cw in enumerate(CHUNKS):
                cs = slice(cstart, cstart + cw)
                cstart += cw
                if c == 0:
                    xhi_t, xlo_t, h8_t, h16_t = xhi0, xlo0, h80, h160
                else:
                    xhi_t = []
                    for kp in range(KP):
                        xm = xp.tile([128, 2 * cw], f8, tag=f"x{kp}",
                                     name=f"xhi{c}_{kp}")
                        nc.sync.dma_start(
                            pair3(xm, cw),
                            xhiT.ap()[kp * 256:(kp + 1) * 256, cs]
                                .rearrange("(k p) b -> p k b", p=128))
                        xhi_t.append(xm)
                    xlo_t = []
                    for kp in range(KP):
                        xm = xp.tile([128, 2 * cw], f8, tag=f"xl{kp}",
                                     name=f"xlo{c}_{kp}")
                        nc.sync.dma_start(
                            pair3(xm, cw),
                            xloT.ap()[kp * 256:(kp + 1) * 256, cs]
                                .rearrange("(k p) b -> p k b", p=128))
                        xlo_t.append(xm)
                    h8_t = hp.tile([128, 2 * cw], f8, tag="h8", name=f"h8{c}")
                    nc.sync.dma_start(
                        pair3(h8_t, cw),
                        h8T.ap()[:, cs].rearrange("(k p) b -> p k b", p=128))
                    h16_t = hp.tile([128, 2 * cw], f16, tag="h16", name=f"h16{c}")
                    nc.sync.dma_start(
                        pair3(h16_t, cw),
                        h16T.ap()[:, cs].rearrange("(k p) b -> p k b", p=128))

                # PSUM accumulators. r/z gates take input-proj + hidden-proj
                # into the same bank (only their sum is needed downstream).
                p_rz = [pp.tile([128, cw], f32, tag=f"p{gt}", name=f"prz{gt}")
                        for gt in range(2 * ST)]
                p_in = [pp.tile([128, cw], f32, tag=f"p{2 * ST + t_}", name=f"pin{t_}")
                        for t_ in range(ST)]
                p_hn = [pp.tile([128, cw], f32, tag=f"p{3 * ST + t_}", name=f"phn{t_}")
                        for t_ in range(ST)]

                def dmm(ps, wt, winner, gt, xt, start, stop):
                    nc.tensor.matmul(
                        ps[:], pair3(wt, winner)[:, :, gsl(gt)],
                        pair3(xt, cw), start=start, stop=stop, perf_mode=DR)

                # Phase 1: w_hi @ x_hi, k-pair-major, all six input psums —
                # PE consumption tracks the whi/xhi DMA arrival order.
                for kp in range(KP):
                    for gt in range(2 * ST):
                        dmm(p_rz[gt], whi_sb[kp], G3, gt, xhi_t[kp],
                            kp == 0, False)
                    for t_ in range(ST):
                        dmm(p_in[t_], whi_sb[kp], G3, 4 + t_, xhi_t[kp],
                            kp == 0, False)
                # Phase 2: w_lo @ x_hi residual pass for the z/n gates
                # (wlo gate-tile j = gate-tile 2+j of the full stationary).
                for kp in range(KP):
                    for t_ in range(ST):
                        dmm(p_rz[ST + t_], wlo_sb[kp], GZN, t_, xhi_t[kp],
                            False, False)
                    for t_ in range(ST):
                        dmm(p_in[t_], wlo_sb[kp], GZN, 2 + t_, xhi_t[kp],
                            False, False)
                # Phase 3: w_hi @ x_lo residual pass for z/n, first KP-1
                # k-pairs (the last lands in the stagger tail).
                for kp in range(KP - 1):
                    for t_ in range(ST):
                        dmm(p_rz[ST + t_], whi_sb[kp], G3, ST + t_, xlo_t[kp],
                            False, False)
                    for t_ in range(ST):
                        dmm(p_in[t_], whi_sb[kp], G3, 4 + t_, xlo_t[kp],
                            False, False)

                kl = KP - 1
                last = (c == len(CHUNKS) - 1)
                o = op.tile([128, ST * cw], f32, tag="o")

                # Stagger tails so psums complete in drain order: r gates
                # first (sigmoids free banks early), i_n last (shortest
                # post-PE chain).
                def r_tail(t_):
                    dmm(p_rz[t_], whm, G3, t_, h8_t, False, True)

                def hn_tail(t_):
                    dmm(p_hn[t_], whm, G3, 4 + t_, h8_t, True, True)

                def z_tail(t_):
                    dmm(p_rz[ST + t_], whi_sb[kl], G3, ST + t_, xlo_t[kl],
                        False, False)
                    dmm(p_rz[ST + t_], whm, G3, ST + t_, h8_t, False, True)

                def in_tail(t_):
                    dmm(p_in[t_], whi_sb[kl], G3, 4 + t_, xlo_t[kl],
                        False, True)

                def ew_r(t_):
                    r = gp.tile([128, cw], f32, tag=f"r{t_}", name=f"r{t_}")
                    nc.scalar.activation(r[:], p_rz[t_][:], ACT.Sigmoid,
                                         bias=brz_sb[:, t_:t_ + 1],
                                         scale=1.0 / WS)
                    a = gp.tile([128, cw], f32, tag=f"a{t_}", name=f"a{t_}")
                    nc.vector.scalar_tensor_tensor(
                        a[:], p_hn[t_][:], bhn_sb[:, t_:t_ + 1], r[:],
                        ALU.add, ALU.mult)
                    return a

                def ew_z(t_):
                    z = gp.tile([128, cw], f32, tag=f"z{t_}", name=f"z{t_}")
                    nc.scalar.activation(z[:], p_rz[ST + t_][:], ACT.Sigmoid,
                                         bias=brz_sb[:, ST + t_:ST + t_ + 1],
                                         scale=1.0 / WS)
                    zc = gp.tile([128, cw], f32, tag=f"zc{t_}", name=f"zc{t_}")
                    nc.gpsimd.tensor_scalar(zc[:], z[:], -1.0, 1.0,
                                            ALU.mult, ALU.add)
                    return z, zc

                def ew_zh(t_, z):
                    zh = gp.tile([128, cw], f32, tag=f"zh{t_}", name=f"zh{t_}")
                    nc.vector.tensor_mul(zh[:], z[:],
                                         h16_t[:, t_ * cw:(t_ + 1) * cw])
                    return zh

                def ew_tanh(t_, a):
                    b2 = gp.tile([128, cw], f32, tag=f"b{t_}", name=f"b{t_}")
                    nc.vector.tensor_add(b2[:], a[:], p_in[t_][:])
                    n_ = gp.tile([128, cw], f32, tag=f"n{t_}", name=f"n{t_}")
                    nc.scalar.activation(n_[:], b2[:], ACT.Tanh,
                                         bias=bin_sb[:, t_:t_ + 1],
                                         scale=1.0 / WS)
                    return n_

                def ew_out(t_, n_, zc, zh):
                    e = gp.tile([128, cw], f32, tag=f"e{t_}", name=f"e{t_}")
                    nc.vector.tensor_mul(e[:], n_[:], zc[:])
                    nc.vector.tensor_add(o[:, t_ * cw:(t_ + 1) * cw], e[:],
                                         zh[:])

                if not last:
                    for t_ in range(ST):
                        r_tail(t_)
                    for t_ in range(ST):
                        hn_tail(t_)
                    for t_ in range(ST):
                        z_tail(t_)
                    for t_ in range(ST):
                        in_tail(t_)
                    as_ = [ew_r(t_) for t_ in range(ST)]
                    zzc = [ew_z(t_) for t_ in range(ST)]
                    zhs = [ew_zh(t_, zzc[t_][0]) for t_ in range(ST)]
                    ns_ = [ew_tanh(t_, as_[t_]) for t_ in range(ST)]
                    for t_ in range(ST):
                        ew_out(t_, ns_[t_], zzc[t_][1], zhs[t_])
                    nc.scalar.dma_start(
                        oT.ap().rearrange("(t p) b -> p t b", p=128)[:, :, cs],
                        o[:].rearrange("p (t c) -> p t c", t=ST))
                else:
                    # final chunk: i_n psums complete before the z-gates so
                    # the b2/tanh chain runs under the last matmuls; b2 goes
                    # ahead of zh on the VectorE queue; per-tile output DMAs
                    # on the scalar and sync DGE queues.
                    for t_ in range(ST):
                        r_tail(t_)
                    for t_ in range(ST):
                        hn_tail(t_)
                    for t_ in range(ST):
                        in_tail(t_)
                    for t_ in range(ST):
                        z_tail(t_)
                    as_ = [ew_r(t_) for t_ in range(ST)]
                    zzc = [ew_z(t_) for t_ in range(ST)]
                    ns_ = [ew_tanh(t_, as_[t_]) for t_ in range(ST)]
                    zhs = [ew_zh(t_, zzc[t_][0]) for t_ in range(ST)]
                    for t_ in range(ST):
                        ew_out(t_, ns_[t_], zzc[t_][1], zhs[t_])
                        eng = nc.scalar if t_ == 0 else nc.sync
                        eng.dma_start(
                            oT.ap()[t_ * 128:(t_ + 1) * 128, cs],
                            o[:, t_ * cw:(t_ + 1) * cw])

    nc.compile()
    return nc


def _get_nc():
    global _cached
    if _cached is None:
        _cached = _build()
    return _cached


def kernel(input, hidden, W_ih, W_hh, b_ih, b_hh):
    input = np.asarray(input, dtype=np.float32)
    hidden = np.asarray(hidden, dtype=np.float32)
    W_ih = np.asarray(W_ih, dtype=np.float32)
    W_hh = np.asarray(W_hh, dtype=np.float32)
    b_ih = np.asarray(b_ih, dtype=np.float32)
    b_hh = np.asarray(b_hh, dtype=np.float32)

    nc = _get_nc()
    from concourse.bass_utils import run_bass_kernel_spmd

    f8 = ml_dtypes.float8_e4m3
    xT = np.ascontiguousarray(input.T)
    xhi8 = xT.astype(f8)
    xlo8 = (xT - xhi8.astype(np.float32)).astype(f8)
    in_maps = []
    for n in range(NUM_BLOCKS):
        brz_n = (b_ih[n, :2 * BS] + b_hh[n, :2 * BS]).reshape(2 * ST, 128).T
        bzn_n = -brz_n[:, ST:]
        bin_n = b_ih[n, 2 * BS:].reshape(ST, 128).T
        bhn_n = (b_hh[n, 2 * BS:] * WS).reshape(ST, 128).T
        bias_n = np.concatenate([brz_n, bzn_n, bin_n, bhn_n], axis=1)
        wT = np.ascontiguousarray(W_ih[n].T) * WS          # [I, G3], scaled
        whi8 = wT.astype(f8)
        wlo8 = (wT[:, BS:] - whi8[:, BS:].astype(np.float32)).astype(f8)
        hTn = np.ascontiguousarray(hidden[:, n * BS:(n + 1) * BS].T)
        in_maps.append({
            "xhiT": xhi8,
            "xloT": xlo8,
            "wihHi": np.ascontiguousarray(whi8),
            "wihLo": np.ascontiguousarray(wlo8),
            "whh": np.ascontiguousarray((W_hh[n].T * WS).astype(f8)),
            "h8T": hTn.astype(f8),
            "h16T": hTn.astype(np.float16),
            "bias": np.ascontiguousarray(bias_n.astype(np.float32)),
        })

    res = run_bass_kernel_spmd(nc, in_maps, core_ids=list(range(NUM_BLOCKS)))
    out = np.empty((BATCH, HIDDEN_DIM), dtype=np.float32)
    for n in range(NUM_BLOCKS):
        out[:, n * BS:(n + 1) * BS] = res.results[n]["oT"].T
    return out


# revision 3
# speedup vs baseline: 1.1723x; 1.0905x over previous
"""BlockGRU Trainium2 kernel.

Block-diagonal GRU cell: 8 independent blocks (block_size 256), batch 2048,
input_dim 1024. Sharded one block per NeuronCore (8 cores).

Per-core layout: gates on partitions, batch on the free dimension
(everything transposed on the host, which is free). Matmul datapath runs
fp8e4m3 with DoubleRow perf mode (two 128-deep k-slices contracted per
instruction at 0.5 cycles/row — 4x the fp16 rate). Accuracy is recovered
with error-compensation passes: activations and weights are split into
fp8 hi + fp8 residual streams (host-side quantization), and the
gate-sensitive z/n input projections accumulate the w_hi@x_lo and
w_lo@x_hi cross terms (the r gate and the short hidden-side projections
tolerate plain fp8; measured end-to-end rel-L2 vs the fp32 reference:
9.2e-3). Weights are pre-scaled by 64 into fp8's normal range; the
descale folds into the ScalarE activation `scale` and a host-prescaled
b_hn. The hidden state is shipped twice: fp8 for the matmul, fp16 for
the elementwise z*h path.

Each chunk's matmuls run in two waves — r/z psums (banks 0-3, drained
by their sigmoids right after wave 1) then hn/in psums (banks 4-7) — so
the next chunk's wave-1 matmuls overlap this chunk's wave 2 without an
accumulation-group conflict, keeping PE busy across chunk boundaries.
PSUM accumulates in fp32; per-partition biases fuse into ScalarE
activation ops; the z*h and n*(1-z) products run on the Pool engine.
"""

import sys

if "/opt/trn_rl_repo" not in sys.path:
    sys.path.insert(0, "/opt/trn_rl_repo")

import numpy as np
import ml_dtypes

INPUT_DIM = 1024
HIDDEN_DIM = 2048
NUM_BLOCKS = 8
BS = HIDDEN_DIM // NUM_BLOCKS  # 256
G3 = 3 * BS                    # 768
GZN = 2 * BS                   # 512 z+n gate columns carrying residual passes
BATCH = 2048
CHUNKS = [512, 512, 512, 256, 256]   # batch chunks (PSUM bank = 512 fp32;
                                     # small tail chunks shorten the post-PE tail)
KP = INPUT_DIM // 256          # 4 DoubleRow k-pairs on the input side
ST = BS // 128                 # 2 state partition-tiles per block
WS = 64.0                      # weight pre-scale into fp8 normal range

_cached = None


def _build():
    import concourse.tile as tile
    import concourse.mybir as mybir
    from concourse import bacc

    f32 = mybir.dt.float32
    f16 = mybir.dt.float16
    f8 = mybir.dt.float8e4
    ALU = mybir.AluOpType
    ACT = mybir.ActivationFunctionType
    DR = mybir.MatmulPerfMode.DoubleRow

    nc = bacc.Bacc("TRN2", target_bir_lowering=False, debug=False, num_devices=8)

    xhiT = nc.dram_tensor("xhiT", [INPUT_DIM, BATCH], f8, kind="ExternalInput")
    xloT = nc.dram_tensor("xloT", [INPUT_DIM, BATCH], f8, kind="ExternalInput")
    wihHi = nc.dram_tensor("wihHi", [INPUT_DIM, G3], f8, kind="ExternalInput")
    wihLo = nc.dram_tensor("wihLo", [INPUT_DIM, GZN], f8, kind="ExternalInput")
    whh = nc.dram_tensor("whh", [BS, G3], f8, kind="ExternalInput")
    h8T = nc.dram_tensor("h8T", [BS, BATCH], f8, kind="ExternalInput")
    h16T = nc.dram_tensor("h16T", [BS, BATCH], f16, kind="ExternalInput")
    bias = nc.dram_tensor("bias", [128, 5 * ST], f32, kind="ExternalInput")
    oT = nc.dram_tensor("oT", [BS, BATCH], f32, kind="ExternalOutput")

    with tile.TileContext(nc) as tc:
        with (
            tc.tile_pool(name="const", bufs=1) as cp,
            tc.tile_pool(name="xin", bufs=3) as xp,
            tc.tile_pool(name="hin", bufs=3) as hp,
            tc.tile_pool(name="gates", bufs=4) as gp,
            tc.tile_pool(name="outs", bufs=3) as op,
            tc.tile_pool(name="psum", bufs=1, space="PSUM") as pp,
        ):
            # PE warm-up: harmless matmuls on a zeroed tile while the prefill
            # DMA runs, so the clock ramp (cold -> full rate) completes before
            # real work arrives. Uses the p0 PSUM slot ahead of chunk 0.
            wu = cp.tile([128, 32], f16, tag="wu")
            nc.vector.memset(wu[:], 0.0)
            pdummy = pp.tile([128, 32], f32, tag="p0", name="pdummy")
            for _ in range(64):
                nc.tensor.matmul(pdummy[0:32, :], wu[:, 0:32], wu[:],
                                 start=True, stop=True)

            # --- DMA prologue on the sync (SP) queue. The queue is serial
            # at HBM bandwidth, so emission order == arrival order == PE
            # consumption order: whi + chunk-0 x_hi first (wave-1 bulk),
            # then wlo, hidden-side weights/state (wave-1 tails), x_lo,
            # biases + fp16 h last (drain-time operands). Each tensor is a
            # single multi-k-slice DMA to keep the SP sequencer cheap. ---
            c0w = CHUNKS[0]
            cs0 = slice(0, c0w)

            def load_k(pool, dram, cols, n_k, dt, tag, name):
                t = pool.tile([128, n_k * cols], dt, tag=tag, name=name)
                nc.sync.dma_start(
                    t[:].rearrange("p (k c) -> p k c", k=n_k),
                    dram.rearrange("(k p) c -> p k c", p=128))
                return t

            whi = load_k(cp, wihHi.ap(), G3, 2 * KP, f8, "whi", "whi")
            xhi0 = load_k(xp, xhiT.ap()[:, cs0], c0w, 2 * KP, f8, "xhi", "xhi0")
            wlo = load_k(cp, wihLo.ap(), GZN, 2 * KP, f8, "wlo", "wlo")
            whm = load_k(cp, whh.ap(), G3, 2, f8, "whm", "whm")
            h80 = load_k(hp, h8T.ap()[:, cs0], c0w, 2, f8, "h8", "h80")
            xlo0 = load_k(xp, xloT.ap()[:, cs0], c0w, 2 * KP, f8, "xlo", "xlo0")
            bias_sb = cp.tile([128, 5 * ST], f32, tag="bias")
            nc.sync.dma_start(bias_sb[:], bias.ap())
            h160 = load_k(hp, h16T.ap()[:, cs0], c0w, 2, f16, "h16", "h160")

            brz_sb = bias_sb[:, 0:2 * ST]
            bin_sb = bias_sb[:, 3 * ST:4 * ST]
            bhn_sb = bias_sb[:, 4 * ST:5 * ST]   # pre-scaled by WS on host

            def gsl(gt):
                return slice(gt * 128, (gt + 1) * 128)

            cstart = 0
            for c, cw in enumerate(CHUNKS):
                cs = slice(cstart, cstart + cw)
                cstart += cw
                if c == 0:
                    xhi_t, xlo_t, h8_t, h16_t = xhi0, xlo0, h80, h160
                else:
                    xhi_t = load_k(xp, xhiT.ap()[:, cs], cw, 2 * KP, f8,
                                   "xhi", f"xhi{c}")
                    xlo_t = load_k(xp, xloT.ap()[:, cs], cw, 2 * KP, f8,
                                   "xlo", f"xlo{c}")
                    h8_t = load_k(hp, h8T.ap()[:, cs], cw, 2, f8, "h8", f"h8{c}")
                    h16_t = load_k(hp, h16T.ap()[:, cs], cw, 2, f16,
                                   "h16", f"h16{c}")

                # PSUM accumulators. r/z gates take input-proj + hidden-proj
                # into the same bank (only their sum is needed downstream).
                # Banks 0-3: r/z (wave 1); banks 4-7: in/hn (wave 2).
                p_rz = [pp.tile([128, cw], f32, tag=f"p{gt}", name=f"prz{gt}")
                        for gt in range(2 * ST)]
                p_in = [pp.tile([128, cw], f32, tag=f"p{2 * ST + t_}", name=f"pin{t_}")
                        for t_ in range(ST)]
                p_hn = [pp.tile([128, cw], f32, tag=f"p{3 * ST + t_}", name=f"phn{t_}")
                        for t_ in range(ST)]

                def dmm(ps, wt, winner, gt, kp, xt, start, stop):
                    nc.tensor.matmul(
                        ps[:],
                        wt[:].rearrange("p (k g) -> p k g", g=winner)
                            [:, 2 * kp:2 * kp + 2, gsl(gt)],
                        xt[:].rearrange("p (k c) -> p k c", c=cw)
                            [:, 2 * kp:2 * kp + 2, :],
                        start=start, stop=stop, perf_mode=DR)

                def h_mm(ps, gt, start, stop):
                    nc.tensor.matmul(
                        ps[:],
                        whm[:].rearrange("p (k g) -> p k g", g=G3)[:, :, gsl(gt)],
                        h8_t[:].rearrange("p (k c) -> p k c", c=cw),
                        start=start, stop=stop, perf_mode=DR)

                def wave_rz():
                    # w_hi @ x_hi bulk, k-pair-major (tracks DMA arrival)
                    for kp in range(KP):
                        for gt in range(2 * ST):
                            dmm(p_rz[gt], whi, G3, gt, kp, xhi_t, kp == 0, False)
                    # z/n residual passes: w_lo@x_hi then w_hi@x_lo
                    for kp in range(KP):
                        for t_ in range(ST):
                            dmm(p_rz[ST + t_], wlo, GZN, t_, kp, xhi_t,
                                False, False)
                    for kp in range(KP - 1):
                        for t_ in range(ST):
                            dmm(p_rz[ST + t_], whi, G3, ST + t_, kp, xlo_t,
                                False, False)
                    # tails: r first (sigmoids drain banks 0-1 earliest)
                    for t_ in range(ST):
                        h_mm(p_rz[t_], t_, False, True)
                    for t_ in range(ST):
                        dmm(p_rz[ST + t_], whi, G3, ST + t_, KP - 1, xlo_t,
                            False, False)
                        h_mm(p_rz[ST + t_], ST + t_, False, True)

                def wave_inhn():
                    # hn first so the r-gate scalar_tensor_tensor starts early
                    for t_ in range(ST):
                        h_mm(p_hn[t_], 4 + t_, True, True)
                    for kp in range(KP):
                        for t_ in range(ST):
                            dmm(p_in[t_], whi, G3, 4 + t_, kp, xhi_t,
                                kp == 0, False)
                    for kp in range(KP):
                        for t_ in range(ST):
                            dmm(p_in[t_], wlo, GZN, 2 + t_, kp, xhi_t,
                                False, False)
                    for kp in range(KP - 1):
                        for t_ in range(ST):
                            dmm(p_in[t_], whi, G3, 4 + t_, kp, xlo_t,
                                False, False)
                    for t_ in range(ST):
                        dmm(p_in[t_], whi, G3, 4 + t_, KP - 1, xlo_t,
                            False, True)

                wave_rz()
                wave_inhn()

                o = op.tile([128, ST * cw], f32, tag="o")

                def ew_r(t_):
                    r = gp.tile([128, cw], f32, tag=f"r{t_}", name=f"r{t_}")
                    nc.scalar.activation(r[:], p_rz[t_][:], ACT.Sigmoid,
                                         bias=brz_sb[:, t_:t_ + 1],
                                         scale=1.0 / WS)
                    a = gp.tile([128, cw], f32, tag=f"a{t_}", name=f"a{t_}")
                    nc.vector.scalar_tensor_tensor(
                        a[:], p_hn[t_][:], bhn_sb[:, t_:t_ + 1], r[:],
                        ALU.add, ALU.mult)
                    return a

                def ew_z(t_):
                    z = gp.tile([128, cw], f32, tag=f"z{t_}", name=f"z{t_}")
                    nc.scalar.activation(z[:], p_rz[ST + t_][:], ACT.Sigmoid,
                                         bias=brz_sb[:, ST + t_:ST + t_ + 1],
                                         scale=1.0 / WS)
                    zc = gp.tile([128, cw], f32, tag=f"zc{t_}", name=f"zc{t_}")
                    nc.gpsimd.tensor_scalar(zc[:], z[:], -1.0, 1.0,
                                            ALU.mult, ALU.add)
                    return z, zc

                def ew_zh(t_, z):
                    zh = gp.tile([128, cw], f32, tag=f"zh{t_}", name=f"zh{t_}")
                    nc.gpsimd.tensor_mul(zh[:], z[:],
                                         h16_t[:, t_ * cw:(t_ + 1) * cw])
                    return zh

                def ew_tanh(t_, a):
                    b2 = gp.tile([128, cw], f32, tag=f"b{t_}", name=f"b{t_}")
                    nc.vector.tensor_add(b2[:], a[:], p_in[t_][:])
                    n_ = gp.tile([128, cw], f32, tag=f"n{t_}", name=f"n{t_}")
                    nc.scalar.activation(n_[:], b2[:], ACT.Tanh,
                                         bias=bin_sb[:, t_:t_ + 1],
                                         scale=1.0 / WS)
                    return n_

                def ew_out(t_, n_, zc, zh):
                    e = gp.tile([128, cw], f32, tag=f"e{t_}", name=f"e{t_}")
                    nc.gpsimd.tensor_mul(e[:], n_[:], zc[:])
                    nc.vector.tensor_add(o[:, t_ * cw:(t_ + 1) * cw], e[:],
                                         zh[:])

                last = (c == len(CHUNKS) - 1)
                as_ = [ew_r(t_) for t_ in range(ST)]
                zzc = [ew_z(t_) for t_ in range(ST)]
                zhs = [ew_zh(t_, zzc[t_][0]) for t_ in range(ST)]
                ns_ = [ew_tanh(t_, as_[t_]) for t_ in range(ST)]
                if not last:
                    for t_ in range(ST):
                        ew_out(t_, ns_[t_], zzc[t_][1], zhs[t_])
                    nc.scalar.dma_start(
                        oT.ap().rearrange("(t p) b -> p t b", p=128)[:, :, cs],
                        o[:].rearrange("p (t c) -> p t c", t=ST))
                else:
                    # final chunk: per-tile output DMAs on the scalar and
                    # sync DGE queues right after each tile's last add.
                    for t_ in range(ST):
                        ew_out(t_, ns_[t_], zzc[t_][1], zhs[t_])
                        eng = nc.scalar if t_ == 0 else nc.sync
                        eng.dma_start(
                            oT.ap()[t_ * 128:(t_ + 1) * 128, cs],
                            o[:, t_ * cw:(t_ + 1) * cw])

    nc.compile()
    return nc


def _get_nc():
    global _cached
    if _cached is None:
        _cached = _build()
    return _cached


def kernel(input, hidden, W_ih, W_hh, b_ih, b_hh):
    input = np.asarray(input, dtype=np.float32)
    hidden = np.asarray(hidden, dtype=np.float32)
    W_ih = np.asarray(W_ih, dtype=np.float32)
    W_hh = np.asarray(W_hh, dtype=np.float32)
    b_ih = np.asarray(b_ih, dtype=np.float32)
    b_hh = np.asarray(b_hh, dtype=np.float32)

    nc = _get_nc()
    from concourse.bass_utils import run_bass_kernel_spmd

    f8 = ml_dtypes.float8_e4m3
    xT = np.ascontiguousarray(input.T)
    xhi8 = xT.astype(f8)
    xlo8 = (xT - xhi8.astype(np.float32)).astype(f8)
    in_maps = []
    for n in range(NUM_BLOCKS):
        brz_n = (b_ih[n, :2 * BS] + b_hh[n, :2 * BS]).reshape(2 * ST, 128).T
        bzn_n = -brz_n[:, ST:]
        bin_n = b_ih[n, 2 * BS:].reshape(ST, 128).T
        bhn_n = (b_hh[n, 2 * BS:] * WS).reshape(ST, 128).T
        bias_n = np.concatenate([brz_n, bzn_n, bin_n, bhn_n], axis=1)
        wT = np.ascontiguousarray(W_ih[n].T) * WS          # [I, G3], scaled
        whi8 = wT.astype(f8)
        wlo8 = (wT[:, BS:] - whi8[:, BS:].astype(np.float32)).astype(f8)
        hTn = np.ascontiguousarray(hidden[:, n * BS:(n + 1) * BS].T)
        in_maps.append({
            "xhiT": xhi8,
            "xloT": xlo8,
            "wihHi": np.ascontiguousarray(whi8),
            "wihLo": np.ascontiguousarray(wlo8),
            "whh": np.ascontiguousarray((W_hh[n].T * WS).astype(f8)),
            "h8T": hTn.astype(f8),
            "h16T": hTn.astype(np.float16),
            "bias": np.ascontiguousarray(bias_n.astype(np.float32)),
        })

    res = run_bass_kernel_spmd(nc, in_maps, core_ids=list(range(NUM_BLOCKS)))
    out = np.empty((BATCH, HIDDEN_DIM), dtype=np.float32)
    for n in range(NUM_BLOCKS):
        out[:, n * BS:(n + 1) * BS] = res.results[n]["oT"].T
    return out


# revision 4
# speedup vs baseline: 1.3378x; 1.1411x over previous
"""BlockGRU Trainium2 kernel.

Block-diagonal GRU cell: 8 independent blocks (block_size 256), batch 2048,
input_dim 1024. Sharded one block per NeuronCore (8 cores).

Per-core layout: gates on partitions, batch on the free dimension
(everything transposed on the host, which is free). Matmul datapath runs
fp8e4m3 with DoubleRow perf mode (two 128-deep k-slices contracted per
instruction at 0.5 cycles/row — 4x the fp16 rate). Accuracy is recovered
with error-compensation passes: activations and weights are split into
fp8 hi + fp8 residual streams (host-side quantization), and the
gate-sensitive z/n input projections accumulate the w_hi@x_lo and
w_lo@x_hi cross terms (the r gate and the short hidden-side projections
tolerate plain fp8; measured end-to-end rel-L2 vs the fp32 reference:
9.2e-3). Weights are pre-scaled by 64 into fp8's normal range; the
descale folds into the ScalarE activation `scale` and a host-prescaled
b_hn. The hidden state is shipped twice: fp8 for the matmul, fp16 for
the elementwise z*h path. x_hi and x_lo ship as one stacked dram tensor
so each chunk's input is a single DMA on the sync queue; h16 and the
output ride the scalar queue.

Each chunk's matmuls run in two waves — r/z psums (banks 0-3, drained
by their sigmoids right after wave 1) then hn/in psums (banks 4-7) — so
the next chunk's wave-1 matmuls overlap this chunk's wave 2 without an
accumulation-group conflict, keeping PE busy across chunk boundaries.
PSUM accumulates in fp32; per-partition biases fuse into ScalarE
activation ops; 1-z and z*h run on the Pool engine (early, off the
critical tail), n*(1-z) and the final add on VectorE.
"""

import sys

if "/opt/trn_rl_repo" not in sys.path:
    sys.path.insert(0, "/opt/trn_rl_repo")

import numpy as np
import ml_dtypes

INPUT_DIM = 1024
HIDDEN_DIM = 2048
NUM_BLOCKS = 8
BS = HIDDEN_DIM // NUM_BLOCKS  # 256
G3 = 3 * BS                    # 768
GZN = 2 * BS                   # 512 z+n gate columns carrying residual passes
BATCH = 2048
CHUNKS = [512, 512, 512, 256, 256]   # batch chunks (PSUM bank = 512 fp32;
                                     # small tail chunks shorten the post-PE tail)
KP = INPUT_DIM // 256          # 4 DoubleRow k-pairs on the input side
ST = BS // 128                 # 2 state partition-tiles per block
WS = 64.0                      # weight pre-scale into fp8 normal range
WARMUP = 112

_cached = None


def _build():
    import concourse.tile as tile
    import concourse.mybir as mybir
    from concourse import bacc

    f32 = mybir.dt.float32
    f16 = mybir.dt.float16
    f8 = mybir.dt.float8e4
    ALU = mybir.AluOpType
    ACT = mybir.ActivationFunctionType
    DR = mybir.MatmulPerfMode.DoubleRow

    nc = bacc.Bacc("TRN2", target_bir_lowering=False, debug=False, num_devices=8)

    x2T = nc.dram_tensor("x2T", [2 * INPUT_DIM, BATCH], f8, kind="ExternalInput")
    wihHi = nc.dram_tensor("wihHi", [INPUT_DIM, G3], f8, kind="ExternalInput")
    wihLo = nc.dram_tensor("wihLo", [INPUT_DIM, GZN], f8, kind="ExternalInput")
    whh = nc.dram_tensor("whh", [BS, G3], f8, kind="ExternalInput")
    h8T = nc.dram_tensor("h8T", [BS, BATCH], f8, kind="ExternalInput")
    h16T = nc.dram_tensor("h16T", [BS, BATCH], f16, kind="ExternalInput")
    bias = nc.dram_tensor("bias", [128, 5 * ST], f32, kind="ExternalInput")
    oT = nc.dram_tensor("oT", [BS, BATCH], f32, kind="ExternalOutput")

    with tile.TileContext(nc) as tc:
        with (
            tc.tile_pool(name="const", bufs=1) as cp,
            tc.tile_pool(name="xin", bufs=3) as xp,
            tc.tile_pool(name="hin", bufs=3) as hp,
            tc.tile_pool(name="gates", bufs=4) as gp,
            tc.tile_pool(name="outs", bufs=3) as op,
            tc.tile_pool(name="psum", bufs=1, space="PSUM") as pp,
        ):
            # PE warm-up: harmless matmuls on a zeroed tile while the prefill
            # DMA runs, so the clock ramp (cold -> full rate) completes before
            # real work arrives. Uses the p0 PSUM slot ahead of chunk 0.
            wu = cp.tile([128, 32], f16, tag="wu")
            nc.vector.memset(wu[:], 0.0)
            pdummy = pp.tile([128, 32], f32, tag="p0", name="pdummy")
            for _ in range(WARMUP):
                nc.tensor.matmul(pdummy[0:32, :], wu[:, 0:32], wu[:],
                                 start=True, stop=True)

            # --- DMA prologue, split across the sync (SP) and scalar (ACT)
            # HWDGE queues. Each queue is serial at its own rate, so emission
            # order == arrival order == PE consumption order. Sync: whi and
            # chunk-0 x (in three slabs so wave 1 starts on the first);
            # scalar: wlo, hidden-side weights/state, biases, fp16 h. ---
            c0w = CHUNKS[0]
            cs0 = slice(0, c0w)

            def load_k(eng, pool, dram, cols, n_k, dt, tag, name):
                t = pool.tile([128, n_k * cols], dt, tag=tag, name=name)
                eng.dma_start(
                    t[:].rearrange("p (k c) -> p k c", k=n_k),
                    dram.rearrange("(k p) c -> p k c", p=128))
                return t

            whi = load_k(nc.sync, cp, wihHi.ap(), G3, 2 * KP, f8, "whi", "whi")
            xm0 = xp.tile([128, 4 * KP * c0w], f8, tag="x2", name="x2_0")
            xm0v = xm0[:].rearrange("p (k c) -> p k c", k=4 * KP)
            for lo, hi in ((0, 4), (4, 8), (8, 16)):
                nc.sync.dma_start(
                    xm0v[:, lo:hi, :],
                    x2T.ap()[lo * 128:hi * 128, cs0]
                        .rearrange("(k p) b -> p k b", p=128))
            wlo = load_k(nc.scalar, cp, wihLo.ap(), GZN, 2 * KP, f8, "wlo", "wlo")
            whm = load_k(nc.scalar, cp, whh.ap(), G3, 2, f8, "whm", "whm")
            h80 = load_k(nc.scalar, hp, h8T.ap()[:, cs0], c0w, 2, f8, "h8", "h80")
            bias_sb = cp.tile([128, 5 * ST], f32, tag="bias")
            nc.scalar.dma_start(bias_sb[:], bias.ap())
            h160 = load_k(nc.scalar, hp, h16T.ap()[:, cs0], c0w, 2, f16,
                          "h16", "h160")

            brz_sb = bias_sb[:, 0:2 * ST]
            bin_sb = bias_sb[:, 3 * ST:4 * ST]
            bhn_sb = bias_sb[:, 4 * ST:5 * ST]   # pre-scaled by WS on host

            def gsl(gt):
                return slice(gt * 128, (gt + 1) * 128)

            cstart = 0
            for c, cw in enumerate(CHUNKS):
                cs = slice(cstart, cstart + cw)
                cstart += cw
                if c == 0:
                    xm, h8_t, h16_t = xm0, h80, h160
                else:
                    xm = load_k(nc.sync, xp, x2T.ap()[:, cs], cw, 4 * KP, f8,
                                "x2", f"x2_{c}")
                    h8_t = load_k(nc.sync, hp, h8T.ap()[:, cs], cw, 2, f8,
                                  "h8", f"h8{c}")
                    h16_t = load_k(nc.scalar, hp, h16T.ap()[:, cs], cw, 2, f16,
                                   "h16", f"h16{c}")

                # PSUM accumulators. r/z gates take input-proj + hidden-proj
                # into the same bank (only their sum is needed downstream).
                # Banks 0-3: r/z (wave 1); banks 4-7: in/hn (wave 2).
                p_rz = [pp.tile([128, cw], f32, tag=f"p{gt}", name=f"prz{gt}")
                        for gt in range(2 * ST)]
                p_in = [pp.tile([128, cw], f32, tag=f"p{2 * ST + t_}", name=f"pin{t_}")
                        for t_ in range(ST)]
                p_hn = [pp.tile([128, cw], f32, tag=f"p{3 * ST + t_}", name=f"phn{t_}")
                        for t_ in range(ST)]

                xv = xm[:].rearrange("p (k c) -> p k c", c=cw)

                def dmm(ps, wt, winner, gt, kp, xlo_side, start, stop):
                    base = 2 * KP if xlo_side else 0
                    nc.tensor.matmul(
                        ps[:],
                        wt[:].rearrange("p (k g) -> p k g", g=winner)
                            [:, 2 * kp:2 * kp + 2, gsl(gt)],
                        xv[:, base + 2 * kp:base + 2 * kp + 2, :],
                        start=start, stop=stop, perf_mode=DR)

                def h_mm(ps, gt, start, stop):
                    nc.tensor.matmul(
                        ps[:],
                        whm[:].rearrange("p (k g) -> p k g", g=G3)[:, :, gsl(gt)],
                        h8_t[:].rearrange("p (k c) -> p k c", c=cw),
                        start=start, stop=stop, perf_mode=DR)

                def wave_rz():
                    # w_hi @ x_hi bulk, k-pair-major (tracks DMA arrival)
                    for kp in range(KP):
                        for gt in range(2 * ST):
                            dmm(p_rz[gt], whi, G3, gt, kp, False, kp == 0, False)
                    # z residual passes: w_lo@x_hi then w_hi@x_lo
                    for kp in range(KP):
                        for t_ in range(ST):
                            dmm(p_rz[ST + t_], wlo, GZN, t_, kp, False,
                                False, False)
                    for kp in range(KP - 1):
                        for t_ in range(ST):
                            dmm(p_rz[ST + t_], whi, G3, ST + t_, kp, True,
                                False, False)
                    # tails: r first (sigmoids drain banks 0-1 earliest)
                    for t_ in range(ST):
                        h_mm(p_rz[t_], t_, False, True)
                    for t_ in range(ST):
                        dmm(p_rz[ST + t_], whi, G3, ST + t_, KP - 1, True,
                            False, False)
                        h_mm(p_rz[ST + t_], ST + t_, False, True)

                def wave_inhn():
                    # hn first so the r-gate scalar_tensor_tensor starts early
                    for t_ in range(ST):
                        h_mm(p_hn[t_], 4 + t_, True, True)
                    for kp in range(KP):
                        for t_ in range(ST):
                            dmm(p_in[t_], whi, G3, 4 + t_, kp, False,
                                kp == 0, False)
                    for kp in range(KP):
                        for t_ in range(ST):
                            dmm(p_in[t_], wlo, GZN, 2 + t_, kp, False,
                                False, False)
                    for kp in range(KP - 1):
                        for t_ in range(ST):
                            dmm(p_in[t_], whi, G3, 4 + t_, kp, True,
                                False, False)
                    for t_ in range(ST):
                        dmm(p_in[t_], whi, G3, 4 + t_, KP - 1, True,
                            False, True)

                wave_rz()
                wave_inhn()

                o = op.tile([128, ST * cw], f32, tag="o")

                def ew_r(t_):
                    r = gp.tile([128, cw], f32, tag=f"r{t_}", name=f"r{t_}")
                    nc.scalar.activation(r[:], p_rz[t_][:], ACT.Sigmoid,
                                         bias=brz_sb[:, t_:t_ + 1],
                                         scale=1.0 / WS)
                    a = gp.tile([128, cw], f32, tag=f"a{t_}", name=f"a{t_}")
                    nc.vector.scalar_tensor_tensor(
                        a[:], p_hn[t_][:], bhn_sb[:, t_:t_ + 1], r[:],
                        ALU.add, ALU.mult)
                    return a

                def ew_z(t_):
                    z = gp.tile([128, cw], f32, tag=f"z{t_}", name=f"z{t_}")
                    nc.scalar.activation(z[:], p_rz[ST + t_][:], ACT.Sigmoid,
                                         bias=brz_sb[:, ST + t_:ST + t_ + 1],
                                         scale=1.0 / WS)
                    zc = gp.tile([128, cw], f32, tag=f"zc{t_}", name=f"zc{t_}")
                    nc.gpsimd.tensor_scalar(zc[:], z[:], -1.0, 1.0,
                                            ALU.mult, ALU.add)
                    return z, zc

                def ew_zh(t_, z):
                    zh = gp.tile([128, cw], f32, tag=f"zh{t_}", name=f"zh{t_}")
                    nc.gpsimd.tensor_mul(zh[:], z[:],
                                         h16_t[:, t_ * cw:(t_ + 1) * cw])
                    return zh

                def ew_tanh(t_, a):
                    b2 = gp.tile([128, cw], f32, tag=f"b{t_}", name=f"b{t_}")
                    nc.vector.tensor_add(b2[:], a[:], p_in[t_][:])
                    n_ = gp.tile([128, cw], f32, tag=f"n{t_}", name=f"n{t_}")
                    nc.scalar.activation(n_[:], b2[:], ACT.Tanh,
                                         bias=bin_sb[:, t_:t_ + 1],
                                         scale=1.0 / WS)
                    return n_

                def ew_out(t_, n_, zc, zh):
                    e = gp.tile([128, cw], f32, tag=f"e{t_}", name=f"e{t_}")
                    nc.vector.tensor_mul(e[:], n_[:], zc[:])
                    nc.vector.tensor_add(o[:, t_ * cw:(t_ + 1) * cw], e[:],
                                         zh[:])

                last = (c == len(CHUNKS) - 1)
                as_ = [ew_r(t_) for t_ in range(ST)]
                zzc = [ew_z(t_) for t_ in range(ST)]
                zhs = [ew_zh(t_, zzc[t_][0]) for t_ in range(ST)]
                ns_ = [ew_tanh(t_, as_[t_]) for t_ in range(ST)]
                if not last:
                    for t_ in range(ST):
                        ew_out(t_, ns_[t_], zzc[t_][1], zhs[t_])
                    nc.scalar.dma_start(
                        oT.ap().rearrange("(t p) b -> p t b", p=128)[:, :, cs],
                        o[:].rearrange("p (t c) -> p t c", t=ST))
                else:
                    # final chunk: per-tile output DMAs on the scalar and
                    # sync DGE queues right after each tile's last add.
                    for t_ in range(ST):
                        ew_out(t_, ns_[t_], zzc[t_][1], zhs[t_])
                        eng = nc.scalar if t_ == 0 else nc.sync
                        eng.dma_start(
                            oT.ap()[t_ * 128:(t_ + 1) * 128, cs],
                            o[:, t_ * cw:(t_ + 1) * cw])

    nc.compile()
    return nc


def _get_nc():
    global _cached
    if _cached is None:
        _cached = _build()
    return _cached


def kernel(input, hidden, W_ih, W_hh, b_ih, b_hh):
    input = np.asarray(input, dtype=np.float32)
    hidden = np.asarray(hidden, dtype=np.float32)
    W_ih = np.asarray(W_ih, dtype=np.float32)
    W_hh = np.asarray(W_hh, dtype=np.float32)
    b_ih = np.asarray(b_ih, dtype=np.float32)
    b_hh = np.asarray(b_hh, dtype=np.float32)

    nc = _get_nc()
    from concourse.bass_utils import run_bass_kernel_spmd

    f8 = ml_dtypes.float8_e4m3
    xT = np.ascontiguousarray(input.T)
    xhi8 = xT.astype(f8)
    xlo8 = (xT - xhi8.astype(np.float32)).astype(f8)
    x2 = np.ascontiguousarray(np.concatenate([xhi8, xlo8], axis=0))
    in_maps = []
    for n in range(NUM_BLOCKS):
        brz_n = (b_ih[n, :2 * BS] + b_hh[n, :2 * BS]).reshape(2 * ST, 128).T
        bzn_n = -brz_n[:, ST:]
        bin_n = b_ih[n, 2 * BS:].reshape(ST, 128).T
        bhn_n = (b_hh[n, 2 * BS:] * WS).reshape(ST, 128).T
        bias_n = np.concatenate([brz_n, bzn_n, bin_n, bhn_n], axis=1)
        wT = np.ascontiguousarray(W_ih[n].T) * WS          # [I, G3], scaled
        whi8 = wT.astype(f8)
        wlo8 = (wT[:, BS:] - whi8[:, BS:].astype(np.float32)).astype(f8)
        hTn = np.ascontiguousarray(hidden[:, n * BS:(n + 1) * BS].T)
        in_maps.append({
            "x2T": x2,
            "wihHi": np.ascontiguousarray(whi8),
            "wihLo": np.ascontiguousarray(wlo8),
            "whh": np.ascontiguousarray((W_hh[n].T * WS).astype(f8)),
            "h8T": hTn.astype(f8),
            "h16T": hTn.astype(np.float16),
            "bias": np.ascontiguousarray(bias_n.astype(np.float32)),
        })

    res = run_bass_kernel_spmd(nc, in_maps, core_ids=list(range(NUM_BLOCKS)))
    out = np.empty((BATCH, HIDDEN_DIM), dtype=np.float32)
    for n in range(NUM_BLOCKS):
        out[:, n * BS:(n + 1) * BS] = res.results[n]["oT"].T
    return out


# revision 8
# speedup vs baseline: 1.3554x; 1.0132x over previous
"""BlockGRU Trainium2 kernel.

Block-diagonal GRU cell: 8 independent blocks (block_size 256), batch 2048,
input_dim 1024. Sharded one block per NeuronCore (8 cores).

Per-core layout: gates on partitions, batch on the free dimension
(everything transposed on the host, which is free). Matmul datapath runs
fp8e4m3 with DoubleRow perf mode (two 128-deep k-slices contracted per
instruction at 0.5 cycles/row — 4x the fp16 rate). Accuracy is recovered
with error-compensation passes: activations and weights are split into
fp8 hi + fp8 residual streams (host-side quantization), and the
gate-sensitive z/n input projections accumulate the w_hi@x_lo and
w_lo@x_hi cross terms (the r gate and the short hidden-side projections
tolerate plain fp8; measured end-to-end rel-L2 vs the fp32 reference:
9.2e-3). Weights are pre-scaled by 64 into fp8's normal range; the
descale folds into the ScalarE activation `scale` and a host-prescaled
b_hn. The hidden state is shipped twice: fp8 for the matmul, fp16 for
the elementwise z*h path. x_hi and x_lo ship as one stacked dram tensor
so each chunk's input is a single DMA on the sync queue; h16 and the
output ride the scalar queue.

Each chunk's matmuls run in two waves — r/z psums (banks 0-3, drained
by their sigmoids right after wave 1) then hn/in psums (banks 4-7) — so
the next chunk's wave-1 matmuls overlap this chunk's wave 2 without an
accumulation-group conflict, keeping PE busy across chunk boundaries.
PSUM accumulates in fp32; per-partition biases fuse into ScalarE
activation ops; 1-z and z*h run on the Pool engine (early, off the
critical tail), n*(1-z) and the final add on VectorE.
"""

import sys

if "/opt/trn_rl_repo" not in sys.path:
    sys.path.insert(0, "/opt/trn_rl_repo")

import numpy as np
import ml_dtypes

INPUT_DIM = 1024
HIDDEN_DIM = 2048
NUM_BLOCKS = 8
BS = HIDDEN_DIM // NUM_BLOCKS  # 256
G3 = 3 * BS                    # 768
GZN = 2 * BS                   # 512 z+n gate columns carrying residual passes
BATCH = 2048
CHUNKS = [512, 512, 512, 256, 256]   # batch chunks (PSUM bank = 512 fp32;
                                     # small tail chunks shorten the post-PE tail)
KP = INPUT_DIM // 256          # 4 DoubleRow k-pairs on the input side
ST = BS // 128                 # 2 state partition-tiles per block
WS = 64.0                      # weight pre-scale into fp8 normal range
WARMUP = 112

_cached = None


def _build():
    import concourse.tile as tile
    import concourse.mybir as mybir
    from concourse import bacc

    f32 = mybir.dt.float32
    f16 = mybir.dt.float16
    f8 = mybir.dt.float8e4
    ALU = mybir.AluOpType
    ACT = mybir.ActivationFunctionType
    DR = mybir.MatmulPerfMode.DoubleRow

    nc = bacc.Bacc("TRN2", target_bir_lowering=False, debug=False, num_devices=8)

    x2T = nc.dram_tensor("x2T", [2 * INPUT_DIM, BATCH], f8, kind="ExternalInput")
    wihHi = nc.dram_tensor("wihHi", [INPUT_DIM, G3], f8, kind="ExternalInput")
    wihLo = nc.dram_tensor("wihLo", [INPUT_DIM, GZN], f8, kind="ExternalInput")
    whh = nc.dram_tensor("whh", [BS, G3], f8, kind="ExternalInput")
    h8T = nc.dram_tensor("h8T", [BS, BATCH], f8, kind="ExternalInput")
    h16T = nc.dram_tensor("h16T", [BS, BATCH], f16, kind="ExternalInput")
    bias = nc.dram_tensor("bias", [128, 5 * ST], f32, kind="ExternalInput")
    oT = nc.dram_tensor("oT", [BS, BATCH], f32, kind="ExternalOutput")

    with tile.TileContext(nc) as tc:
        with (
            tc.tile_pool(name="const", bufs=1) as cp,
            tc.tile_pool(name="xin", bufs=3) as xp,
            tc.tile_pool(name="hin", bufs=3) as hp,
            tc.tile_pool(name="gates", bufs=4) as gp,
            tc.tile_pool(name="outs", bufs=3) as op,
            tc.tile_pool(name="psum", bufs=1, space="PSUM") as pp,
        ):
            # PE warm-up: harmless matmuls on a zeroed tile while the prefill
            # DMA runs, so the clock ramp (cold -> full rate) completes before
            # real work arrives. Uses the p0 PSUM slot ahead of chunk 0.
            wu = cp.tile([128, 32], f16, tag="wu")
            nc.vector.memset(wu[:], 0.0)
            pdummy = pp.tile([128, 32], f32, tag="p0", name="pdummy")
            for _ in range(WARMUP):
                nc.tensor.matmul(pdummy[0:32, :], wu[:, 0:32], wu[:],
                                 start=True, stop=True)

            # --- DMA prologue, split across the sync (SP) and scalar (ACT)
            # HWDGE queues. Each queue is serial at its own rate, so emission
            # order == arrival order == PE consumption order. Sync: whi and
            # chunk-0 x (in three slabs so wave 1 starts on the first);
            # scalar: wlo, hidden-side weights/state, biases, fp16 h. ---
            c0w = CHUNKS[0]
            cs0 = slice(0, c0w)

            def load_k(eng, pool, dram, cols, n_k, dt, tag, name):
                t = pool.tile([128, n_k * cols], dt, tag=tag, name=name)
                eng.dma_start(
                    t[:].rearrange("p (k c) -> p k c", k=n_k),
                    dram.rearrange("(k p) c -> p k c", p=128))
                return t

            whi = cp.tile([128, 2 * KP * G3], f8, tag="whi", name="whi")
            whiv = whi[:].rearrange("p (k g) -> p k g", k=2 * KP)
            nc.sync.dma_start(
                whiv[:, 0:4, :],
                wihHi.ap()[0:512, :].rearrange("(k p) g -> p k g", p=128))
            nc.scalar.dma_start(
                whiv[:, 4:8, :],
                wihHi.ap()[512:1024, :].rearrange("(k p) g -> p k g", p=128))
            xm0 = xp.tile([128, 4 * KP * c0w], f8, tag="x2", name="x2_0")
            xm0v = xm0[:].rearrange("p (k c) -> p k c", k=4 * KP)
            for lo, hi in ((0, 4), (4, 8), (8, 16)):
                nc.sync.dma_start(
                    xm0v[:, lo:hi, :],
                    x2T.ap()[lo * 128:hi * 128, cs0]
                        .rearrange("(k p) b -> p k b", p=128))
            wlo = load_k(nc.scalar, cp, wihLo.ap(), GZN, 2 * KP, f8, "wlo", "wlo")
            whm = load_k(nc.scalar, cp, whh.ap(), G3, 2, f8, "whm", "whm")
            h80 = load_k(nc.scalar, hp, h8T.ap()[:, cs0], c0w, 2, f8, "h8", "h80")
            bias_sb = cp.tile([128, 5 * ST], f32, tag="bias")
            nc.scalar.dma_start(bias_sb[:], bias.ap())
            h160 = load_k(nc.scalar, hp, h16T.ap()[:, cs0], c0w, 2, f16,
                          "h16", "h160")

            brz_sb = bias_sb[:, 0:2 * ST]
            bzn_sb = bias_sb[:, 2 * ST:3 * ST]   # -brz[z]: 1-z = sigmoid(-u)
            bin_sb = bias_sb[:, 3 * ST:4 * ST]
            bhn_sb = bias_sb[:, 4 * ST:5 * ST]   # pre-scaled by WS on host

            def gsl(gt):
                return slice(gt * 128, (gt + 1) * 128)

            cstart = 0
            for c, cw in enumerate(CHUNKS):
                cs = slice(cstart, cstart + cw)
                cstart += cw
                if c == 0:
                    xm, h8_t, h16_t = xm0, h80, h160
                else:
                    xm = load_k(nc.sync, xp, x2T.ap()[:, cs], cw, 4 * KP, f8,
                                "x2", f"x2_{c}")
                    h8_t = load_k(nc.sync, hp, h8T.ap()[:, cs], cw, 2, f8,
                                  "h8", f"h8{c}")
                    h16_t = load_k(nc.scalar, hp, h16T.ap()[:, cs], cw, 2, f16,
                                   "h16", f"h16{c}")

                # PSUM accumulators. r/z gates take input-proj + hidden-proj
                # into the same bank (only their sum is needed downstream).
                # Banks 0-3: r/z (wave 1); banks 4-7: in/hn (wave 2).
                p_rz = [pp.tile([128, cw], f32, tag=f"p{gt}", name=f"prz{gt}")
                        for gt in range(2 * ST)]
                p_in = [pp.tile([128, cw], f32, tag=f"p{2 * ST + t_}", name=f"pin{t_}")
                        for t_ in range(ST)]
                p_hn = [pp.tile([128, cw], f32, tag=f"p{3 * ST + t_}", name=f"phn{t_}")
                        for t_ in range(ST)]

                xv = xm[:].rearrange("p (k c) -> p k c", c=cw)

                def dmm(ps, wt, winner, gt, kp, xlo_side, start, stop):
                    base = 2 * KP if xlo_side else 0
                    nc.tensor.matmul(
                        ps[:],
                        wt[:].rearrange("p (k g) -> p k g", g=winner)
                            [:, 2 * kp:2 * kp + 2, gsl(gt)],
                        xv[:, base + 2 * kp:base + 2 * kp + 2, :],
                        start=start, stop=stop, perf_mode=DR)

                def h_mm(ps, gt, start, stop):
                    nc.tensor.matmul(
                        ps[:],
                        whm[:].rearrange("p (k g) -> p k g", g=G3)[:, :, gsl(gt)],
                        h8_t[:].rearrange("p (k c) -> p k c", c=cw),
                        start=start, stop=stop, perf_mode=DR)

                def wave_rz():
                    # w_hi @ x_hi bulk, k-pair-major (tracks DMA arrival)
                    for kp in range(KP):
                        for gt in range(2 * ST):
                            dmm(p_rz[gt], whi, G3, gt, kp, False, kp == 0, False)
                    # z residual passes: w_lo@x_hi then w_hi@x_lo
                    for kp in range(KP):
                        for t_ in range(ST):
                            dmm(p_rz[ST + t_], wlo, GZN, t_, kp, False,
                                False, False)
                    for kp in range(KP - 1):
                        for t_ in range(ST):
                            dmm(p_rz[ST + t_], whi, G3, ST + t_, kp, True,
                                False, False)
                    # tails: r first (sigmoids drain banks 0-1 earliest)
                    for t_ in range(ST):
                        h_mm(p_rz[t_], t_, False, True)
                    for t_ in range(ST):
                        dmm(p_rz[ST + t_], whi, G3, ST + t_, KP - 1, True,
                            False, False)
                        h_mm(p_rz[ST + t_], ST + t_, False, True)

                def wave_inhn():
                    # hn first so the r-gate scalar_tensor_tensor starts early
                    for t_ in range(ST):
                        h_mm(p_hn[t_], 4 + t_, True, True)
                    for kp in range(KP):
                        for t_ in range(ST):
                            dmm(p_in[t_], whi, G3, 4 + t_, kp, False,
                                kp == 0, False)
                    for kp in range(KP):
                        for t_ in range(ST):
                            dmm(p_in[t_], wlo, GZN, 2 + t_, kp, False,
                                False, False)
                    for kp in range(KP - 1):
                        for t_ in range(ST):
                            dmm(p_in[t_], whi, G3, 4 + t_, kp, True,
                                False, False)
                    for t_ in range(ST):
                        dmm(p_in[t_], whi, G3, 4 + t_, KP - 1, True,
                            False, True)

                wave_rz()
                wave_inhn()

                o = op.tile([128, ST * cw], f32, tag="o")

                def ew_r(t_):
                    r = gp.tile([128, cw], f32, tag=f"r{t_}", name=f"r{t_}")
                    nc.scalar.activation(r[:], p_rz[t_][:], ACT.Sigmoid,
                                         bias=brz_sb[:, t_:t_ + 1],
                                         scale=1.0 / WS)
                    a = gp.tile([128, cw], f32, tag=f"a{t_}", name=f"a{t_}")
                    nc.vector.scalar_tensor_tensor(
                        a[:], p_hn[t_][:], bhn_sb[:, t_:t_ + 1], r[:],
                        ALU.add, ALU.mult)
                    return a

                def ew_z(t_, act_zc):
                    z = gp.tile([128, cw], f32, tag=f"z{t_}", name=f"z{t_}")
                    nc.scalar.activation(z[:], p_rz[ST + t_][:], ACT.Sigmoid,
                                         bias=brz_sb[:, ST + t_:ST + t_ + 1],
                                         scale=1.0 / WS)
                    zc = gp.tile([128, cw], f32, tag=f"zc{t_}", name=f"zc{t_}")
                    if act_zc:
                        # 1-z = sigmoid(-u): second read of the z psum on
                        # ScalarE keeps the Pool engine off the final tail
                        nc.scalar.activation(zc[:], p_rz[ST + t_][:],
                                             ACT.Sigmoid,
                                             bias=bzn_sb[:, t_:t_ + 1],
                                             scale=-1.0 / WS)
                    else:
                        nc.gpsimd.tensor_scalar(zc[:], z[:], -1.0, 1.0,
                                                ALU.mult, ALU.add)
                    return z, zc

                def ew_zh(t_, z):
                    zh = gp.tile([128, cw], f32, tag=f"zh{t_}", name=f"zh{t_}")
                    nc.gpsimd.tensor_mul(zh[:], z[:],
                                         h16_t[:, t_ * cw:(t_ + 1) * cw])
                    return zh

                def ew_tanh(t_, a):
                    b2 = gp.tile([128, cw], f32, tag=f"b{t_}", name=f"b{t_}")
                    nc.vector.tensor_add(b2[:], a[:], p_in[t_][:])
                    n_ = gp.tile([128, cw], f32, tag=f"n{t_}", name=f"n{t_}")
                    nc.scalar.activation(n_[:], b2[:], ACT.Tanh,
                                         bias=bin_sb[:, t_:t_ + 1],
                                         scale=1.0 / WS)
                    return n_

                def ew_out(t_, n_, zc, zh):
                    e = gp.tile([128, cw], f32, tag=f"e{t_}", name=f"e{t_}")
                    nc.vector.tensor_mul(e[:], n_[:], zc[:])
                    nc.vector.tensor_add(o[:, t_ * cw:(t_ + 1) * cw], e[:],
                                         zh[:])

                last = (c == len(CHUNKS) - 1)
                act_zc = c >= len(CHUNKS) - 2
                as_ = [ew_r(t_) for t_ in range(ST)]
                zzc = [ew_z(t_, act_zc) for t_ in range(ST)]
                zhs = [ew_zh(t_, zzc[t_][0]) for t_ in range(ST)]
                ns_ = [ew_tanh(t_, as_[t_]) for t_ in range(ST)]
                if not last:
                    for t_ in range(ST):
                        ew_out(t_, ns_[t_], zzc[t_][1], zhs[t_])
                    nc.scalar.dma_start(
                        oT.ap().rearrange("(t p) b -> p t b", p=128)[:, :, cs],
                        o[:].rearrange("p (t c) -> p t c", t=ST))
                else:
                    # final chunk: per-tile output DMAs on the scalar and
                    # sync DGE queues right after each tile's last add.
                    for t_ in range(ST):
                        ew_out(t_, ns_[t_], zzc[t_][1], zhs[t_])
                        eng = nc.scalar if t_ == 0 else nc.sync
                        eng.dma_start(
                            oT.ap()[t_ * 128:(t_ + 1) * 128, cs],
                            o[:, t_ * cw:(t_ + 1) * cw])

    nc.compile()
    return nc


def _get_nc():
    global _cached
    if _cached is None:
        _cached = _build()
    return _cached


def kernel(input, hidden, W_ih, W_hh, b_ih, b_hh):
    input = np.asarray(input, dtype=np.float32)
    hidden = np.asarray(hidden, dtype=np.float32)
    W_ih = np.asarray(W_ih, dtype=np.float32)
    W_hh = np.asarray(W_hh, dtype=np.float32)
    b_ih = np.asarray(b_ih, dtype=np.float32)
    b_hh = np.asarray(b_hh, dtype=np.float32)

    nc = _get_nc()
    from concourse.bass_utils import run_bass_kernel_spmd

    f8 = ml_dtypes.float8_e4m3
    xT = np.ascontiguousarray(input.T)
    xhi8 = xT.astype(f8)
    xlo8 = (xT - xhi8.astype(np.float32)).astype(f8)
    x2 = np.ascontiguousarray(np.concatenate([xhi8, xlo8], axis=0))
    in_maps = []
    for n in range(NUM_BLOCKS):
        brz_n = (b_ih[n, :2 * BS] + b_hh[n, :2 * BS]).reshape(2 * ST, 128).T
        bzn_n = -brz_n[:, ST:]
        bin_n = b_ih[n, 2 * BS:].reshape(ST, 128).T
        bhn_n = (b_hh[n, 2 * BS:] * WS).reshape(ST, 128).T
        bias_n = np.concatenate([brz_n, bzn_n, bin_n, bhn_n], axis=1)
        wT = np.ascontiguousarray(W_ih[n].T) * WS          # [I, G3], scaled
        whi8 = wT.astype(f8)
        wlo8 = (wT[:, BS:] - whi8[:, BS:].astype(np.float32)).astype(f8)
        hTn = np.ascontiguousarray(hidden[:, n * BS:(n + 1) * BS].T)
        in_maps.append({
            "x2T": x2,
            "wihHi": np.ascontiguousarray(whi8),
            "wihLo": np.ascontiguousarray(wlo8),
            "whh": np.ascontiguousarray((W_hh[n].T * WS).astype(f8)),
            "h8T": hTn.astype(f8),
            "h16T": hTn.astype(np.float16),
            "bias": np.ascontiguousarray(bias_n.astype(np.float32)),
        })

    res = run_bass_kernel_spmd(nc, in_maps, core_ids=list(range(NUM_BLOCKS)))
    out = np.empty((BATCH, HIDDEN_DIM), dtype=np.float32)
    for n in range(NUM_BLOCKS):
        out[:, n * BS:(n + 1) * BS] = res.results[n]["oT"].T
    return out
